# revision 4
# baseline (speedup 1.0000x reference)
"""Trainium2 Bass kernel for LinearMemoryAttention (B=1, S=4096, D=512, H=8, Dh=64).

Sharding: sequence-parallel over 8 cores (512 rows each), all heads local.
Cross-core causal state is resolved with one small AllGather of per-block
(sigma(k) outer v, sigma(k)) sums plus a per-core prefix mask.

Self-contained: hardcodes all shapes; builds/compiles the Bass program once.
"""

import numpy as np

import concourse.bass as bass
import concourse.bacc as bacc
import concourse.mybir as mybir
import concourse.tile as tile
from concourse.bass_utils import run_bass_kernel_spmd

F32 = mybir.dt.float32
N_CORES = 8
S = 4096
D = 512
H = 8
DH = 64
HP = 65  # head width incl. denominator column
S_BLK = S // N_CORES  # 512 rows per core
NCH = S_BLK // 128  # 4 chunks of 128
NHP = H // 2  # 4 head pairs
EPS = 1e-6

_CACHE = {}


def _build():
    Alu = mybir.AluOpType
    Act = mybir.ActivationFunctionType
    nc = bacc.Bacc("TRN2", target_bir_lowering=False, debug=False,
                   num_devices=N_CORES)

    hs_d = nc.dram_tensor("hs", [S_BLK, D], F32, kind="ExternalInput").ap()
    wq_d = nc.dram_tensor("wq", [D, D], F32, kind="ExternalInput").ap()
    wk_d = nc.dram_tensor("wk", [D, D], F32, kind="ExternalInput").ap()
    wv_d = nc.dram_tensor("wv", [D, D], F32, kind="ExternalInput").ap()
    wo_d = nc.dram_tensor("wo", [D, D], F32, kind="ExternalInput").ap()
    bqt_d = nc.dram_tensor("bqt", [128, NHP], F32, kind="ExternalInput").ap()
    bkt_d = nc.dram_tensor("bkt", [128, NHP], F32, kind="ExternalInput").ap()
    bk_d = nc.dram_tensor("bkr", [1, D], F32, kind="ExternalInput").ap()
    bv_d = nc.dram_tensor("bvr", [1, D], F32, kind="ExternalInput").ap()
    mz_d = nc.dram_tensor("mz", [128, NHP * HP], F32, kind="ExternalInput").ap()
    pm_d = nc.dram_tensor("pmask", [128, N_CORES], F32, kind="ExternalInput").ap()
    y_d = nc.dram_tensor("y", [S_BLK, D], F32, kind="ExternalOutput").ap()

    with tile.TileContext(nc) as tc:
        with (
            tc.tile_pool(name="const", bufs=1) as cpool,
            tc.tile_pool(name="wpool", bufs=1) as wpool,
            tc.tile_pool(name="data", bufs=1) as dpool,
            tc.tile_pool(name="tmp", bufs=3) as tpool,
            tc.tile_pool(name="small", bufs=4) as spool,
            tc.tile_pool(name="dram", bufs=1, space="DRAM") as drpool,
        ):
            # ---- constants -------------------------------------------------
            ones128 = cpool.tile([128, 128], F32)
            nc.gpsimd.memset(ones128[:], 1.0)
            ident = cpool.tile([128, 128], F32)
            nc.gpsimd.affine_select(ident[:], ones128[:], pattern=[[1, 128]],
                                    compare_op=Alu.is_equal, fill=0.0, base=0,
                                    channel_multiplier=-1)
            triu = cpool.tile([128, 128], F32)
            nc.gpsimd.affine_select(triu[:], ones128[:], pattern=[[1, 128]],
                                    compare_op=Alu.is_ge, fill=0.0, base=0,
                                    channel_multiplier=-1)
            ones1 = cpool.tile([1, 128], F32)
            nc.gpsimd.memset(ones1[:], 1.0)

            mz = cpool.tile([128, NHP * HP], F32)
            nc.sync.dma_start(mz[:], mz_d[:])
            pmask = cpool.tile([128, N_CORES], F32)
            nc.sync.dma_start(pmask[:], pm_d[:])
            bqt = cpool.tile([128, NHP], F32)
            nc.sync.dma_start(bqt[:], bqt_d[:])
            bkt = cpool.tile([128, NHP], F32)
            nc.sync.dma_start(bkt[:], bkt_d[:])
            bkr = cpool.tile([1, D], F32)
            nc.sync.dma_start(bkr[:], bk_d[:])
            bvr = cpool.tile([1, D], F32)
            nc.sync.dma_start(bvr[:], bv_d[:])

            # ---- weight / input loads -------------------------------------
            wq_t = [wpool.tile([128, D], F32, name=f"wq{i}") for i in range(4)]
            wk_t = [wpool.tile([128, D], F32, name=f"wk{i}") for i in range(4)]
            wv_t = [wpool.tile([128, D], F32, name=f"wv{i}") for i in range(4)]
            wo_t = [wpool.tile([64, D], F32, name=f"wo{i}") for i in range(H)]
            for i in range(4):
                sl = slice(i * 128, (i + 1) * 128)
                nc.sync.dma_start(wq_t[i][:], wq_d[sl, :])
                nc.sync.dma_start(wk_t[i][:], wk_d[sl, :])
                nc.sync.dma_start(wv_t[i][:], wv_d[sl, :])
            for h in range(H):
                nc.sync.dma_start(wo_t[h][:], wo_d[h * 64:(h + 1) * 64, :])
            hs_t = [dpool.tile([128, D], F32, name=f"hs{i}") for i in range(4)]
            for i in range(4):
                nc.sync.dma_start(hs_t[i][:], hs_d[i * 128:(i + 1) * 128, :])

            with tc.tile_pool(name="ps1", bufs=1, space="PSUM") as ps1:
                # ---- hsT via PE transpose ---------------------------------
                hsT = [dpool.tile([128, S_BLK], F32, name=f"hsT{i}")
                       for i in range(4)]
                for st in range(4):
                    for dt in range(4):
                        pst = ps1.tile([128, 128], F32, name="pstr", bufs=2)
                        nc.tensor.transpose(
                            pst[:], hs_t[st][:, dt * 128:(dt + 1) * 128],
                            ident[:])
                        nc.scalar.copy(hsT[dt][:, st * 128:(st + 1) * 128],
                                       pst[:])

                # ---- k/v natural projections + elu ------------------------
                sk_sb = [dpool.tile([128, D], F32, name=f"sk{i}")
                         for i in range(4)]
                v_sb = [dpool.tile([128, H * HP], F32, name=f"v{i}")
                        for i in range(4)]
                for st in range(4):
                    psk = ps1.tile([128, D], F32, name="psproj", bufs=2)
                    for dt in range(4):
                        nc.tensor.matmul(
                            psk[:], hsT[dt][:, st * 128:(st + 1) * 128],
                            wk_t[dt][:], start=(dt == 0), stop=False)
                    nc.tensor.matmul(psk[:], ones1[:], bkr[:],
                                     start=False, stop=True)
                    e_t = tpool.tile([128, D], F32, name="elu_e")
                    r_t = tpool.tile([128, D], F32, name="elu_r")
                    nc.scalar.activation(e_t[:], psk[:], Act.Exp)
                    nc.scalar.activation(r_t[:], psk[:], Act.Relu)
                    nc.vector.scalar_tensor_tensor(
                        sk_sb[st][:], e_t[:], 1.0, r_t[:],
                        op0=Alu.min, op1=Alu.add)

                    psv = ps1.tile([128, D], F32, name="psproj", bufs=2)
                    for dt in range(4):
                        nc.tensor.matmul(
                            psv[:], hsT[dt][:, st * 128:(st + 1) * 128],
                            wv_t[dt][:], start=(dt == 0), stop=False)
                    nc.tensor.matmul(psv[:], ones1[:], bvr[:],
                                     start=False, stop=True)
                    v3 = v_sb[st].rearrange("p (h e) -> p h e", e=HP)
                    nc.vector.tensor_copy(
                        v3[:, :, 0:DH],
                        psv.rearrange("p (h e) -> p h e", e=DH))
                    nc.gpsimd.memset(v3[:, :, DH:HP], 1.0)

                # ---- per-block state sums U, local prefixes L -------------
                L_sb = [None] + [dpool.tile([128, NHP * HP], F32, name=f"L{c}")
                                 for c in range(1, NCH)]
                ball = dpool.tile([128, NHP * HP], F32, name="ball")
                psL = [ps1.tile([128, HP], F32, name=f"psL{hp}")
                       for hp in range(NHP)]
                for c in range(NCH):
                    for hp in range(NHP):
                        for sub in range(2):
                            h = 2 * hp + sub
                            nc.tensor.matmul(
                                psL[hp][sub * 64:(sub + 1) * 64, :],
                                sk_sb[c][:, h * DH:(h + 1) * DH],
                                v_sb[c][:, h * HP:(h + 1) * HP],
                                start=(c == 0), stop=True,
                                tile_position=(0, 64 * sub))
                        dest = L_sb[c + 1] if c < NCH - 1 else ball
                        nc.vector.tensor_copy(
                            dest[:, hp * HP:(hp + 1) * HP], psL[hp][:])

                # ---- collective: allgather block sums ---------------------
                cc_in = drpool.tile([128, NHP * HP], F32)
                cc_out = drpool.tile([N_CORES, 128, NHP * HP], F32,
                                     addr_space="Shared")
                nc.sync.dma_start(cc_in[:], ball[:])
                nc.gpsimd.collective_compute(
                    "AllGather", Alu.bypass,
                    replica_groups=[list(range(N_CORES))],
                    ins=[cc_in[:]], outs=[cc_out[:]])

                # ---- qT / kT projections + elu (overlap collective) -------
                sqT = [dpool.tile([128, S_BLK], F32, name=f"sqT{hp}")
                       for hp in range(NHP)]
                skT = [dpool.tile([128, S_BLK], F32, name=f"skT{hp}")
                       for hp in range(NHP)]
                for hp in range(NHP):
                    for (wt, bias, dst) in ((wq_t, bqt, sqT), (wk_t, bkt, skT)):
                        psq = ps1.tile([128, S_BLK], F32, name="psproj", bufs=2)
                        for dt in range(4):
                            nc.tensor.matmul(
                                psq[:],
                                wt[dt][:, hp * 128:(hp + 1) * 128],
                                hsT[dt][:], start=(dt == 0), stop=(dt == 3))
                        e_t = tpool.tile([128, S_BLK], F32, name="elu_e")
                        r_t = tpool.tile([128, S_BLK], F32, name="elu_r")
                        nc.scalar.activation(e_t[:], psq[:], Act.Exp,
                                             bias=bias[:, hp:hp + 1])
                        nc.scalar.activation(r_t[:], psq[:], Act.Relu,
                                             bias=bias[:, hp:hp + 1])
                        nc.vector.scalar_tensor_tensor(
                            dst[hp][:], e_t[:], 1.0, r_t[:],
                            op0=Alu.min, op1=Alu.add)

                # ---- prefix state P = mz + sum_{j<me} Bg_j ----------------
                bg = dpool.tile([128, N_CORES * NHP * HP], F32, name="bg")
                nc.sync.dma_start(
                    bg.rearrange("p (j e) -> p j e", j=N_CORES),
                    cc_out.rearrange("j p e -> p j e"))
                pa = dpool.tile([128, NHP * HP], F32, name="pa")
                pb = dpool.tile([128, NHP * HP], F32, name="pb")
                acc_src = mz
                W = NHP * HP
                for j in range(N_CORES - 1):
                    acc_dst = pa if j % 2 == 0 else pb
                    nc.vector.scalar_tensor_tensor(
                        acc_dst[:], bg[:, j * W:(j + 1) * W],
                        pmask[:, j:j + 1], acc_src[:],
                        op0=Alu.mult, op1=Alu.add)
                    acc_src = acc_dst
                P = acc_src
                G_sb = [P]
                for c in range(1, NCH):
                    g = dpool.tile([128, NHP * HP], F32, name=f"G{c}")
                    nc.vector.tensor_add(g[:], P[:], L_sb[c][:])
                    G_sb.append(g)

            # ---- phase B: per-chunk per-head attention --------------------
            attnT = [dpool.tile([64, S_BLK], F32, name=f"attnT{h}")
                     for h in range(H)]
            with tc.tile_pool(name="ps2", bufs=1, space="PSUM") as ps2:
                for c in range(NCH):
                    cs = slice(c * 128, (c + 1) * 128)
                    for h in range(H):
                        hp, sub = h // 2, h % 2
                        hb = slice(sub * 64, (sub + 1) * 64)
                        psA = ps2.tile([128, 128], F32, name="psA", bufs=2)
                        nc.tensor.matmul(psA[:], skT[hp][hb, cs],
                                         sqT[hp][hb, cs], start=True,
                                         stop=True)
                        amask = tpool.tile([128, 128], F32, name="amask")
                        nc.vector.tensor_mul(amask[:], psA[:], triu[:])
                        psN = ps2.tile([128, HP], F32, name="psN", bufs=2)
                        nc.tensor.matmul(psN[:], amask[:],
                                         v_sb[c][:, h * HP:(h + 1) * HP],
                                         start=True, stop=False)
                        nc.tensor.matmul(
                            psN[:], sqT[hp][hb, cs],
                            G_sb[c][hb, hp * HP:(hp + 1) * HP],
                            start=False, stop=True)
                        den = spool.tile([128, 1], F32, name="den")
                        nc.vector.tensor_scalar_add(den[:], psN[:, DH:HP], EPS)
                        rec = spool.tile([128, 1], F32, name="rec")
                        nc.vector.reciprocal(rec[:], den[:])
                        attn = tpool.tile([128, DH], F32, name="attn")
                        nc.vector.tensor_scalar_mul(attn[:], psN[:, 0:DH],
                                                    rec[:])
                        psT = ps2.tile([64, 128], F32, name="psT", bufs=2)
                        nc.tensor.transpose(psT[:], attn[:], ident[:])
                        nc.scalar.copy(attnT[h][:, cs], psT[:])

                # ---- output projection ------------------------------------
                for st in range(4):
                    ss = slice(st * 128, (st + 1) * 128)
                    psO = ps2.tile([128, D], F32, name="psO", bufs=2)
                    for h in range(H):
                        nc.tensor.matmul(
                            psO[:], attnT[h][:, ss], wo_t[h][:],
                            start=(h == 0), stop=(h == H - 1))
                    y_sb = tpool.tile([128, D], F32, name="ysb", bufs=2)
                    nc.vector.tensor_copy(y_sb[:], psO[:])
                    nc.sync.dma_start(y_d[ss, :], y_sb[:])

    nc.compile()
    return nc


def _get_nc():
    if "nc" not in _CACHE:
        _CACHE["nc"] = _build()
    return _CACHE["nc"]


def kernel(hidden_states, Wq, bq, Wk, bk, Wv, bv, Wo, M_mem, z_mem):
    nc = _get_nc()
    hs = np.asarray(hidden_states, np.float32).reshape(S, D)
    Wq = np.ascontiguousarray(np.asarray(Wq, np.float32))
    Wk = np.ascontiguousarray(np.asarray(Wk, np.float32))
    Wv = np.ascontiguousarray(np.asarray(Wv, np.float32))
    Wo = np.ascontiguousarray(np.asarray(Wo, np.float32))
    bq = np.asarray(bq, np.float32)
    bk = np.asarray(bk, np.float32)
    bv = np.asarray(bv, np.float32)
    M_mem = np.asarray(M_mem, np.float32)
    z_mem = np.asarray(z_mem, np.float32)

    # per-hpair transposed bias layout [128, NHP]
    bqt = np.ascontiguousarray(bq.reshape(NHP, 128).T)
    bkt = np.ascontiguousarray(bk.reshape(NHP, 128).T)
    bkr = bk.reshape(1, D)
    bvr = bv.reshape(1, D)

    # memory seed [128, NHP*HP]: head h -> rows (h%2)*64.., cols (h//2)*65..
    mz = np.zeros((128, NHP * HP), np.float32)
    for h in range(H):
        pr, col = (h % 2) * 64, (h // 2) * HP
        mz[pr:pr + 64, col:col + DH] = M_mem[h]
        mz[pr:pr + 64, col + DH] = z_mem[h]

    in_maps = []
    for c in range(N_CORES):
        pm = np.zeros((128, N_CORES), np.float32)
        pm[:, :c] = 1.0
        in_maps.append({
            "hs": np.ascontiguousarray(hs[c * S_BLK:(c + 1) * S_BLK]),
            "wq": Wq, "wk": Wk, "wv": Wv, "wo": Wo,
            "bqt": bqt, "bkt": bkt, "bkr": bkr, "bvr": bvr,
            "mz": mz, "pmask": pm,
        })

    res = run_bass_kernel_spmd(nc, in_maps, list(range(N_CORES)))
    out = np.concatenate([res.results[c]["y"] for c in range(N_CORES)], axis=0)
    return out.reshape(1, S, D)
